# revision 8
# baseline (speedup 1.0000x reference)
"""Trainium2 Bass kernel for nn_EncoderLayer_73315091743398.

The reference module's attention einsums ('hwink,hwijm->hwinm') sum their k/j
indices independently, so the whole attention block collapses to, per
(h,w)-chunk c and head i, over the flat q matrix qf = x@Wq.T + pe viewed as
(8192, 512) in raw (s,h,w) row order:

    u[s]  = sum_d qf[c*512+s, 64i+d]          (segment row sums)
    a     = softmax_s(u)
    v[d]  = sum_s a[s] * qf[c*512+s, 64i+d]
    row   = tile8(v) @ Wfc.T = v @ M,  M[d,:] = sum_b Wfc[:, 64b+d].T

and attn_out viewed (S,H,W,D) has row A[s'] = row_{c=s'//32, i=(s'%32)//4},
independent of (h,w).  Core k owns raw rows [k*1024,(k+1)*1024): these are
exactly attention chunks {2k, 2k+1} AND the residual/FFN rows for
s' in [64k, 64k+64), so the 8 cores run fully independent SPMD programs
(data-parallel over the flat row dimension; no collectives).
"""

import math
import os
import sys
from contextlib import ExitStack

import numpy as np

for _p in ("/opt/trn_rl_repo", "/root/.axon_site/_ro/trn_rl_repo"):
    if os.path.isdir(_p) and _p not in sys.path:
        sys.path.append(_p)

import concourse.bass as bass
import concourse.bacc as bacc
import concourse.mybir as mybir
import concourse.tile as tile
from concourse.bass_utils import run_bass_kernel_spmd

F32 = mybir.dt.float32
F32R = mybir.dt.float32r
AF = mybir.ActivationFunctionType
ALU = mybir.AluOpType
AX = mybir.AxisListType

S, H, W, D = 512, 4, 4, 512
NH, DEP, DFF = 8, 64, 2048
NCORES = 8
R = 1024          # rows per core of the flat (8192, 512) view
EPS = 1e-5

_cached = {}


def _r32(ap):
    return ap.bitcast(F32R)


def build_nc():
    """Build the single-core SPMD Bass/Tile program (same program on all 8)."""
    nc = bacc.Bacc("TRN2", debug=False, target_bir_lowering=False)

    xT = nc.dram_tensor("xT", [D, R], F32R, kind="ExternalInput")
    xR = nc.dram_tensor("xR", [R, D], F32, kind="ExternalInput")
    peR = nc.dram_tensor("peR", [R, D], F32, kind="ExternalInput")
    WqT = nc.dram_tensor("WqT", [D, D], F32R, kind="ExternalInput")
    W1T = nc.dram_tensor("W1T", [D, DFF], F32R, kind="ExternalInput")
    W2T = nc.dram_tensor("W2T", [DFF, D], F32R, kind="ExternalInput")
    Mstk = nc.dram_tensor("Mstk", [128, D], F32R, kind="ExternalInput")
    eye = nc.dram_tensor("eye", [128, 128], F32, kind="ExternalInput")
    E2d = nc.dram_tensor("E2", [2, 128], F32, kind="ExternalInput")
    ones1 = nc.dram_tensor("ones1", [128, 1], F32, kind="ExternalInput")
    zer2 = nc.dram_tensor("zer2", [128, 2], F32R, kind="ExternalInput")
    b1c = nc.dram_tensor("b1c", [16, 128, 1], F32, kind="ExternalInput")
    B2r = nc.dram_tensor("B2r", [128, D], F32, kind="ExternalInput")
    G1r = nc.dram_tensor("G1r", [128, D], F32, kind="ExternalInput")
    BE1r = nc.dram_tensor("BE1r", [128, D], F32, kind="ExternalInput")
    G2r = nc.dram_tensor("G2r", [128, D], F32, kind="ExternalInput")
    BE2r = nc.dram_tensor("BE2r", [128, D], F32, kind="ExternalInput")
    out = nc.dram_tensor("out", [R, D], F32, kind="ExternalOutput")

    with ExitStack() as ctx:
        tc = ctx.enter_context(tile.TileContext(nc))
        cst = ctx.enter_context(tc.tile_pool(name="cst", bufs=1))
        xp = ctx.enter_context(tc.tile_pool(name="xp", bufs=1))
        qp = ctx.enter_context(tc.tile_pool(name="qp", bufs=1))
        wk = ctx.enter_context(tc.tile_pool(name="wk", bufs=2))
        ps = ctx.enter_context(tc.tile_pool(name="ps", bufs=1, space="PSUM"))

        def csttile(src, shape, name, dt=F32):
            t = cst.tile(shape, dt, tag=name, name=name)
            nc.sync.dma_start(t[:], src)
            return t

        eye_sb = csttile(eye[:], [128, 128], "eye")
        E2_sb = csttile(E2d[:], [2, 128], "E2")
        on1_sb = csttile(ones1[:], [128, 1], "on1")
        zer2_sb = csttile(zer2[:], [128, 2], "zer2", dt=F32R)
        Mst_sb = csttile(Mstk[:], [128, D], "Mst", dt=F32R)
        B2_sb = csttile(B2r[:], [128, D], "B2")
        G1_sb = csttile(G1r[:], [128, D], "G1")
        BE1_sb = csttile(BE1r[:], [128, D], "BE1")
        G2_sb = csttile(G2r[:], [128, D], "G2")
        BE2_sb = csttile(BE2r[:], [128, D], "BE2")
        b1_sb = [csttile(b1c[ft], [128, 1], f"b1_{ft}") for ft in range(16)]
        epsT = cst.tile([128, 1], F32, tag="eps")
        nc.vector.memset(epsT[:], EPS)

        wq_sb = [csttile(WqT[dt * 128:(dt + 1) * 128, :], [128, D], f"wq{dt}", dt=F32R)
                 for dt in range(4)]
        w1_sb = [csttile(W1T[dt * 128:(dt + 1) * 128, :], [128, DFF], f"w1{dt}", dt=F32R)
                 for dt in range(4)]
        w2_sb = [csttile(W2T[ft * 128:(ft + 1) * 128, :], [128, D], f"w2{ft}", dt=F32R)
                 for ft in range(16)]
        # xT tiles double as o1T tiles later (tag-shared slots, WAR-ordered).
        xT_sb = [xp.tile([128, R], F32R, tag=f"dT{dt}", name=f"xT{dt}") for dt in range(4)]
        for dt in range(4):
            nc.sync.dma_start(xT_sb[dt][:], xT[dt * 128:(dt + 1) * 128, :])

        q_sb = [qp.tile([128, D], F32, tag=f"q{m}", name=f"qsb{m}") for m in range(8)]
        o1_sb = [qp.tile([128, D], F32, tag=f"o1{m}", name=f"o1sb{m}") for m in range(8)]
        uT = [qp.tile([8, D], F32, tag=f"uT{c}", name=f"uTsb{c}") for c in range(2)]

        def layernorm(dst, zin, g_t, be_t):
            """dst = LN(zin) * g + be, per 128-row tile; zin is SBUF f32."""
            ssum = wk.tile([128, 1], F32, tag="ls")
            nc.vector.tensor_reduce(ssum[:], zin[:], axis=AX.X, op=ALU.add)
            mu = wk.tile([128, 1], F32, tag="lm")
            nc.vector.tensor_scalar_mul(mu[:], ssum[:], 1.0 / D)
            sqd = wk.tile([128, D], F32, tag="lq")
            ssq = wk.tile([128, 1], F32, tag="lsq")
            nc.scalar.activation(sqd[:], zin[:], AF.Square, accum_out=ssq[:])
            msq = wk.tile([128, 1], F32, tag="lms")
            nc.vector.tensor_scalar_mul(msq[:], ssq[:], 1.0 / D)
            mu2 = wk.tile([128, 1], F32, tag="lm2")
            nc.vector.tensor_mul(mu2[:], mu[:], mu[:])
            var = wk.tile([128, 1], F32, tag="lv")
            nc.vector.tensor_sub(var[:], msq[:], mu2[:])
            sd = wk.tile([128, 1], F32, tag="lsd")
            nc.scalar.activation(sd[:], var[:], AF.Sqrt, bias=epsT[:, :])
            rsd = wk.tile([128, 1], F32, tag="lr")
            nc.vector.reciprocal(rsd[:], sd[:])
            nrm = wk.tile([128, D], F32, tag="ln")
            nc.vector.tensor_scalar(nrm[:], zin[:], mu[:], rsd[:],
                                    op0=ALU.subtract, op1=ALU.mult)
            nc.vector.tensor_mul(nrm[:], nrm[:], g_t[:])
            nc.vector.tensor_add(dst[:], nrm[:], be_t[:])

        def q_stage(m):
            qps = ps.tile([128, D], F32, tag="mmA", bufs=2)
            for dt in range(4):
                nc.tensor.matmul(qps[:], xT_sb[dt][:, m * 128:(m + 1) * 128],
                                 wq_sb[dt][:],
                                 start=(dt == 0), stop=(dt == 3))
            pet = wk.tile([128, D], F32, tag="pe")
            nc.sync.dma_start(pet[:], peR[m * 128:(m + 1) * 128, :])
            nc.vector.tensor_add(q_sb[m][:], qps[:], pet[:])
            useg = wk.tile([128, 8], F32, tag="useg")
            nc.vector.tensor_reduce(
                useg[:], q_sb[m][:].rearrange("p (h d) -> p h d", h=8),
                axis=AX.X, op=ALU.add)
            utp = ps.tile([8, 128], F32, tag="tp", bufs=2)
            nc.tensor.transpose(utp[:], useg[:], eye_sb[:])
            c, st = divmod(m, 4)
            nc.vector.tensor_copy(uT[c][:, st * 128:(st + 1) * 128], utp[:])

        def attn_stage(c):
            # softmax over s for the 8 heads of chunk c
            mx = wk.tile([8, 1], F32, tag="mx")
            nc.vector.tensor_reduce(mx[:], uT[c][:], axis=AX.X, op=ALU.max)
            nmx = wk.tile([8, 1], F32, tag="nmx")
            nc.vector.tensor_scalar_mul(nmx[:], mx[:], -1.0)
            ex = wk.tile([8, D], F32, tag="ex")
            ssum = wk.tile([8, 1], F32, tag="esum")
            nc.scalar.activation(ex[:], uT[c][:], AF.Exp, bias=nmx[:, :],
                                 accum_out=ssum[:])
            rcp = wk.tile([8, 1], F32, tag="ercp")
            nc.vector.reciprocal(rcp[:], ssum[:])
            a_t = ex
            nc.vector.tensor_scalar_mul(a_t[:], ex[:], rcp[:])

            # prod[st] = q tile (x) broadcast(aT)  --  (128, 8, 64) views
            prods = []
            for st in range(4):
                atp = ps.tile([128, 8], F32, tag="tp", bufs=2)
                nc.tensor.transpose(atp[:], a_t[:, st * 128:(st + 1) * 128],
                                    eye_sb[:8, :8])
                aTs = wk.tile([128, 8], F32, tag=f"aT{st}", bufs=1)
                nc.vector.tensor_copy(aTs[:], atp[:])
                prod = wk.tile([128, D], F32, tag=f"prod{st}", bufs=1)
                nc.vector.tensor_tensor(
                    prod[:].rearrange("p (h d) -> p h d", h=8),
                    q_sb[c * 4 + st][:].rearrange("p (h d) -> p h d", h=8),
                    aTs[:].unsqueeze(-1).broadcast_to([128, 8, 64]),
                    op=ALU.mult)
                prods.append(prod)

            for jt in range(4):
                m = c * 4 + jt
                vc = ps.tile([128, 1], F32, tag="vc", bufs=1)
                for st in range(4):
                    nc.tensor.matmul(vc[:], prods[st][:, jt * 128:(jt + 1) * 128],
                                     on1_sb[:], start=(st == 0), stop=(st == 3))
                vm = wk.tile([128, 2], F32R, tag="vm")
                nc.vector.tensor_copy(vm[:], zer2_sb[:])
                nc.vector.tensor_copy(vm[0:64, 0:1], vc[0:64, :])
                nc.vector.tensor_copy(vm[64:128, 1:2], vc[64:128, :])
                lap = ps.tile([2, D], F32, tag="tp", bufs=2)
                nc.tensor.matmul(lap[:], vm[:], Mst_sb[:],
                                 start=True, stop=True)
                las = wk.tile([2, D], F32, tag="las")
                nc.vector.tensor_copy(las[:], lap[:])
                bcp = ps.tile([128, D], F32, tag="mmB", bufs=2)
                nc.tensor.matmul(bcp[:], E2_sb[:], las[:], start=True, stop=True)
                xrt = wk.tile([128, D], F32, tag="xr")
                nc.sync.dma_start(xrt[:], xR[m * 128:(m + 1) * 128, :])
                z1 = wk.tile([128, D], F32, tag="z1")
                nc.vector.tensor_add(z1[:], bcp[:], xrt[:])
                layernorm(o1_sb[m], z1, G1_sb, BE1_sb)

        for m in range(4):
            q_stage(m)
        attn_stage(0)
        for m in range(4, 8):
            q_stage(m)
        attn_stage(1)

        # transpose o1 -> o1T (reusing the xT slots)
        o1T = [xp.tile([128, R], F32R, tag=f"dT{dt}", name=f"o1T{dt}") for dt in range(4)]
        for m in range(8):
            for dt in range(4):
                tps = ps.tile([128, 128], F32, tag="tp", bufs=2)
                nc.tensor.transpose(tps[:], o1_sb[m][:, dt * 128:(dt + 1) * 128],
                                    eye_sb[:])
                nc.vector.tensor_copy(o1T[dt][:, m * 128:(m + 1) * 128], tps[:])

        # FFN over r-quarters of 256
        for rq in range(4):
            h1s = []
            for ft in range(16):
                p1 = ps.tile([128, 256], F32, tag="mmA", bufs=2)
                for dt in range(4):
                    nc.tensor.matmul(
                        p1[:], w1_sb[dt][:, ft * 128:(ft + 1) * 128],
                        o1T[dt][:, rq * 256:(rq + 1) * 256],
                        start=(dt == 0), stop=(dt == 3))
                h1 = wk.tile([128, 256], F32R, tag=f"h1_{ft}", bufs=1)
                nc.scalar.activation(h1[:], p1[:], AF.Relu, bias=b1_sb[ft][:, :])
                h1s.append(h1)
            for rm in range(2):
                m = rq * 2 + rm
                p2 = ps.tile([128, D], F32, tag="mmB", bufs=2)
                for ft in range(16):
                    nc.tensor.matmul(
                        p2[:], h1s[ft][:, rm * 128:(rm + 1) * 128],
                        w2_sb[ft][:],
                        start=(ft == 0), stop=(ft == 15))
                z2 = wk.tile([128, D], F32, tag="z2")
                nc.vector.tensor_add(z2[:], p2[:], o1_sb[m][:])
                z3 = z2
                nc.vector.tensor_add(z3[:], z2[:], B2_sb[:])
                yt = wk.tile([128, D], F32, tag="yt")
                layernorm(yt, z3, G2_sb, BE2_sb)
                nc.sync.dma_start(out[m * 128:(m + 1) * 128, :], yt[:])

    nc.compile()
    return nc


def _round_f32r(a):
    b = np.ascontiguousarray(a, dtype=np.float32).view(np.uint32)
    out = (b + 0x7FF + ((b >> 12) & 1)) & np.uint32(0xFFFFF000)
    return out.view(np.float32)


def _pe_table():
    pos = np.arange(S, dtype=np.float32)[:, None]
    div = np.exp(np.arange(0, D, 2, dtype=np.float32) * (-math.log(10000.0) / D))
    ang = pos * div
    pe = np.zeros((S, D), np.float32)
    pe[:, 0::2] = np.sin(ang)
    pe[:, 1::2] = np.cos(ang)
    return pe


def make_in_maps(x, Wq, Wfc, W1, b1, W2, b2, g1, be1, g2, be2):
    f32 = lambda a: np.ascontiguousarray(a, dtype=np.float32)
    xf = f32(x).reshape(S * H * W, D)
    pe = _pe_table()
    M = f32(Wfc).reshape(D, NH, DEP).sum(axis=1).T          # (64, 512)
    Mstk = np.concatenate([M, M], axis=0)                   # (128, 512)
    E2 = np.zeros((2, 128), np.float32)
    E2[0, :64] = 1.0
    E2[1, 64:] = 1.0
    shared = dict(
        WqT=_round_f32r(Wq.T), W1T=_round_f32r(W1.T), W2T=_round_f32r(W2.T),
        Mstk=_round_f32r(Mstk), eye=np.eye(128, dtype=np.float32), E2=E2,
        ones1=np.ones((128, 1), np.float32),
        zer2=np.zeros((128, 2), np.float32),
        b1c=f32(b1).reshape(16, 128, 1),
        B2r=f32(np.tile(b2, (128, 1))),
        G1r=f32(np.tile(g1, (128, 1))), BE1r=f32(np.tile(be1, (128, 1))),
        G2r=f32(np.tile(g2, (128, 1))), BE2r=f32(np.tile(be2, (128, 1))),
    )
    maps = []
    for k in range(NCORES):
        sl = xf[k * R:(k + 1) * R]
        m = dict(shared)
        m["xT"] = _round_f32r(sl.T)
        m["xR"] = np.ascontiguousarray(sl)
        m["peR"] = np.ascontiguousarray(np.repeat(pe[k * 64:(k + 1) * 64], 16, axis=0))
        maps.append(m)
    return maps


def kernel(x, Wq, Wfc, W1, b1, W2, b2, g1, be1, g2, be2, _results_hook=None,
           _trace=False, _tmpdir=None):
    if "nc" not in _cached:
        _cached["nc"] = build_nc()
    nc = _cached["nc"]
    in_maps = make_in_maps(x, Wq, Wfc, W1, b1, W2, b2, g1, be1, g2, be2)
    res = run_bass_kernel_spmd(nc, in_maps, list(range(NCORES)),
                               trace=_trace, tmpdir=_tmpdir)
    if _results_hook is not None:
        _results_hook(res)
    y = np.concatenate([res.results[k]["out"] for k in range(NCORES)], axis=0)
    return y.reshape(S, H, W, D)


# revision 12
# speedup vs baseline: 1.2446x; 1.2446x over previous
"""Trainium2 Bass kernel for nn_EncoderLayer_73315091743398.

The reference module's attention einsums ('hwink,hwijm->hwinm') sum their k/j
indices independently, so the whole attention block collapses to, per
(h,w)-chunk c and head i, over the flat q matrix qf = x@Wq.T + pe viewed as
(8192, 512) in raw (s,h,w) row order:

    u[s]  = sum_d qf[c*512+s, 64i+d]          (segment row sums)
    a     = softmax_s(u)
    v[d]  = sum_s a[s] * qf[c*512+s, 64i+d]
    row   = tile8(v) @ Wfc.T = v @ M,  M[d,:] = sum_b Wfc[:, 64b+d].T

and attn_out viewed (S,H,W,D) has row A[s'] = row_{c=s'//32, i=(s'%32)//4},
independent of (h,w).  Core k owns raw rows [k*1024,(k+1)*1024): these are
exactly attention chunks {2k, 2k+1} AND the residual/FFN rows for
s' in [64k, 64k+64), so the 8 cores run fully independent SPMD programs
(data-parallel over the flat row dimension; no collectives).
"""

import math
import os
import sys
from contextlib import ExitStack

import numpy as np

for _p in ("/opt/trn_rl_repo", "/root/.axon_site/_ro/trn_rl_repo"):
    if os.path.isdir(_p) and _p not in sys.path:
        sys.path.append(_p)

import concourse.bass as bass
import concourse.bacc as bacc
import concourse.mybir as mybir
import concourse.tile as tile
from concourse.bass_utils import run_bass_kernel_spmd

F32 = mybir.dt.float32
F32R = mybir.dt.float32r
AF = mybir.ActivationFunctionType
ALU = mybir.AluOpType
AX = mybir.AxisListType

S, H, W, D = 512, 4, 4, 512
NH, DEP, DFF = 8, 64, 2048
NCORES = 8
R = 1024          # rows per core of the flat (8192, 512) view
EPS = 1e-5

# packed fp32 constant block column offsets
O_EYE, O_ON1, O_B2, O_G1, O_BE1, O_G2, O_BE2, O_B1G = (
    0, 128, 129, 641, 1153, 1665, 2177, 2689)
NCF = 2705
# packed f32r constant block column offsets
O_MST, O_ZER, O_ONR, O_E8 = 0, 512, 528, 529
NCR = 1041

_cached = {}


def build_nc():
    """Build the single-core SPMD Bass/Tile program (same program on all 8)."""
    nc = bacc.Bacc("TRN2", debug=False, target_bir_lowering=False)

    xT = nc.dram_tensor("xT", [D, R], F32R, kind="ExternalInput")
    xR = nc.dram_tensor("xR", [R, D], F32, kind="ExternalInput")
    peR = nc.dram_tensor("peR", [R, D], F32, kind="ExternalInput")
    WqT = nc.dram_tensor("WqT", [D, D], F32R, kind="ExternalInput")
    W1T = nc.dram_tensor("W1T", [D, DFF], F32R, kind="ExternalInput")
    W2T = nc.dram_tensor("W2T", [DFF, D], F32R, kind="ExternalInput")
    CF = nc.dram_tensor("CF", [128, NCF], F32, kind="ExternalInput")
    CR = nc.dram_tensor("CR", [128, NCR], F32R, kind="ExternalInput")
    out = nc.dram_tensor("out", [R, D], F32, kind="ExternalOutput")

    with ExitStack() as ctx:
        tc = ctx.enter_context(tile.TileContext(nc))
        cst = ctx.enter_context(tc.tile_pool(name="cst", bufs=1))
        xp = ctx.enter_context(tc.tile_pool(name="xp", bufs=1))
        qp = ctx.enter_context(tc.tile_pool(name="qp", bufs=1))
        wk = ctx.enter_context(tc.tile_pool(name="wk", bufs=2))
        ps = ctx.enter_context(tc.tile_pool(name="ps", bufs=1, space="PSUM"))

        # ---- loads, cheapest-needed-first so PE can start early ----
        xT_all = xp.tile([128, 4 * R], F32R, tag="dT", name="xT_all")
        nc.sync.dma_start(xT_all[:].rearrange("p (t r) -> p t r", t=4),
                          xT.rearrange("(t p) r -> p t r", p=128))
        wq_all = cst.tile([128, 4 * D], F32R, tag="wq", name="wq_all")
        nc.sync.dma_start(wq_all[:].rearrange("p (t j) -> p t j", t=4),
                          WqT.rearrange("(t p) j -> p t j", p=128))
        cf = cst.tile([128, NCF], F32, tag="cf", name="cf")
        nc.sync.dma_start(cf[:], CF[:])
        cr = cst.tile([128, NCR], F32R, tag="cr", name="cr")
        nc.sync.dma_start(cr[:], CR[:])
        w1_all = cst.tile([128, 4 * DFF], F32R, tag="w1", name="w1_all")
        nc.sync.dma_start(w1_all[:].rearrange("p (t j) -> p t j", t=4),
                          W1T.rearrange("(t p) j -> p t j", p=128))
        w2_all = cst.tile([128, 16 * D], F32R, tag="w2", name="w2_all")
        nc.sync.dma_start(w2_all[:].rearrange("p (t j) -> p t j", t=16),
                          W2T.rearrange("(t p) j -> p t j", p=128))

        eye_sb = cf[:, O_EYE:O_EYE + 128]
        on1r = cr[:, O_ONR:O_ONR + 1]
        Mst_sb = cr[:, O_MST:O_MST + D]
        zer8 = cr[:, O_ZER:O_ZER + 8]
        B2_sb = cf[:, O_B2:O_B2 + D]
        G1_sb = cf[:, O_G1:O_G1 + D]
        BE1_sb = cf[:, O_BE1:O_BE1 + D]
        G2_sb = cf[:, O_G2:O_G2 + D]
        BE2_sb = cf[:, O_BE2:O_BE2 + D]
        epsT = cst.tile([128, 1], F32, tag="eps", name="epsT")
        nc.vector.memset(epsT[:], EPS)

        q_sb = [qp.tile([128, D], F32, tag=f"q{m}", name=f"qsb{m}") for m in range(8)]
        o1_sb = [qp.tile([128, D], F32, tag=f"o1{m}", name=f"o1sb{m}") for m in range(8)]
        uT = [qp.tile([8, D], F32, tag=f"uT{c}", name=f"uTsb{c}") for c in range(2)]

        def layernorm(dst, zin, g_t, be_t):
            """dst = LN(zin) * g + be for a 128-row tile (zin SBUF f32)."""
            ssum = wk.tile([128, 1], F32, tag="ls")
            nc.vector.tensor_reduce(ssum[:], zin[:], axis=AX.X, op=ALU.add)
            mu = wk.tile([128, 1], F32, tag="lm")
            nc.vector.tensor_scalar_mul(mu[:], ssum[:], 1.0 / D)
            sqd = wk.tile([128, D], F32, tag="lq")
            ssq = wk.tile([128, 1], F32, tag="lsq")
            nc.scalar.activation(sqd[:], zin[:], AF.Square, accum_out=ssq[:])
            msq = wk.tile([128, 1], F32, tag="lms")
            nc.vector.tensor_scalar_mul(msq[:], ssq[:], 1.0 / D)
            mu2 = wk.tile([128, 1], F32, tag="lm2")
            nc.vector.tensor_mul(mu2[:], mu[:], mu[:])
            var = wk.tile([128, 1], F32, tag="lv")
            nc.vector.tensor_sub(var[:], msq[:], mu2[:])
            sd = wk.tile([128, 1], F32, tag="lsd")
            nc.scalar.activation(sd[:], var[:], AF.Sqrt, bias=epsT[:, :])
            rsd = wk.tile([128, 1], F32, tag="lr")
            nc.vector.reciprocal(rsd[:], sd[:])
            nrm = wk.tile([128, D], F32, tag="ln")
            nc.vector.tensor_scalar(nrm[:], zin[:], mu[:], rsd[:],
                                    op0=ALU.subtract, op1=ALU.mult)
            nc.vector.tensor_mul(nrm[:], nrm[:], g_t[:])
            nc.gpsimd.tensor_add(dst[:], nrm[:], be_t[:])

        def q_stage(m):
            qps = ps.tile([128, D], F32, tag="mmA", bufs=2)
            for dt in range(4):
                nc.tensor.matmul(
                    qps[:], xT_all[:, dt * R + m * 128:dt * R + (m + 1) * 128],
                    wq_all[:, dt * D:(dt + 1) * D],
                    start=(dt == 0), stop=(dt == 3))
            pet = wk.tile([128, D], F32, tag="pe")
            nc.gpsimd.dma_start(pet[:], peR[m * 128:(m + 1) * 128, :])
            nc.vector.tensor_add(q_sb[m][:], qps[:], pet[:])
            useg = wk.tile([128, 8], F32, tag="useg")
            nc.vector.tensor_reduce(
                useg[:], q_sb[m][:].rearrange("p (h d) -> p h d", h=8),
                axis=AX.X, op=ALU.add)
            utp = ps.tile([8, 128], F32, tag="tp", bufs=2)
            nc.tensor.transpose(utp[:], useg[:], eye_sb)
            c, st = divmod(m, 4)
            nc.vector.tensor_copy(uT[c][:, st * 128:(st + 1) * 128], utp[:])

        def attn_stage(c):
            # softmax over s for the 8 heads of chunk c
            mx = wk.tile([8, 1], F32, tag="mx")
            nc.vector.tensor_reduce(mx[:], uT[c][:], axis=AX.X, op=ALU.max)
            nmx = wk.tile([8, 1], F32, tag="nmx")
            nc.vector.tensor_scalar_mul(nmx[:], mx[:], -1.0)
            ex = wk.tile([8, D], F32, tag="ex")
            ssum = wk.tile([8, 1], F32, tag="esum")
            nc.scalar.activation(ex[:], uT[c][:], AF.Exp, bias=nmx[:, :],
                                 accum_out=ssum[:])
            rcp = wk.tile([8, 1], F32, tag="ercp")
            nc.vector.reciprocal(rcp[:], ssum[:])
            a_t = ex
            nc.vector.tensor_scalar_mul(a_t[:], ex[:], rcp[:])

            # prod[st] = q tile * broadcast(aT); vcat = column sums via ones-mm
            vca = ps.tile([1, D], F32, tag="vc", bufs=2)
            for st in range(4):
                atp = ps.tile([128, 8], F32, tag="tp", bufs=2)
                nc.tensor.transpose(atp[:], a_t[:, st * 128:(st + 1) * 128],
                                    eye_sb[:8, :8])
                aTs = wk.tile([128, 8], F32, tag=f"aT{st}", bufs=1)
                nc.vector.tensor_copy(aTs[:], atp[:])
                prod = wk.tile([128, D], F32R, tag=f"prod{st}", bufs=1)
                nc.vector.tensor_tensor(
                    prod[:].rearrange("p (h d) -> p h d", h=8),
                    q_sb[c * 4 + st][:].rearrange("p (h d) -> p h d", h=8),
                    aTs[:].unsqueeze(-1).broadcast_to([128, 8, 64]),
                    op=ALU.mult)
                nc.tensor.matmul(vca[:], on1r, prod[:],
                                 start=(st == 0), stop=(st == 3))
            vcs = wk.tile([1, D], F32, tag="vcs")
            nc.vector.tensor_copy(vcs[:], vca[:])

            # Vm[p, 2t+e] = vcat[128t+p] * (p//64 == e);  La = Vm.T @ Mstk
            vm = wk.tile([128, 8], F32R, tag="vm")
            nc.vector.tensor_copy(vm[:], zer8)
            for tt in range(4):
                vtp = ps.tile([128, 1], F32, tag="tp", bufs=2)
                nc.tensor.transpose(vtp[:], vcs[:, tt * 128:(tt + 1) * 128],
                                    eye_sb[:1, :1])
                nc.vector.tensor_copy(vm[0:64, 2 * tt:2 * tt + 1], vtp[0:64, :])
                nc.vector.tensor_copy(vm[64:128, 2 * tt + 1:2 * tt + 2],
                                      vtp[64:128, :])
            lap = ps.tile([8, D], F32, tag="vc", bufs=2)
            nc.tensor.matmul(lap[:], vm[:], Mst_sb, start=True, stop=True)
            las = wk.tile([8, D], F32R, tag="las")
            nc.vector.tensor_copy(las[:], lap[:])

            for jt in range(4):
                m = c * 4 + jt
                bcp = ps.tile([128, D], F32, tag="mmB", bufs=2)
                nc.tensor.matmul(bcp[:],
                                 cr[0:8, O_E8 + jt * 128:O_E8 + (jt + 1) * 128],
                                 las[:], start=True, stop=True)
                xrt = wk.tile([128, D], F32, tag="xr")
                nc.gpsimd.dma_start(xrt[:], xR[m * 128:(m + 1) * 128, :])
                z1 = wk.tile([128, D], F32, tag="z1")
                nc.vector.tensor_add(z1[:], bcp[:], xrt[:])
                layernorm(o1_sb[m], z1, G1_sb, BE1_sb)

        for m in range(4):
            q_stage(m)
        attn_stage(0)
        for m in range(4, 8):
            q_stage(m)
        attn_stage(1)

        # transpose o1 -> o1T (reusing the xT slots)
        o1T = xp.tile([128, 4 * R], F32R, tag="dT", name="o1T_all")
        for m in range(8):
            for dt in range(4):
                tps = ps.tile([128, 128], F32, tag="tp", bufs=2)
                nc.tensor.transpose(tps[:], o1_sb[m][:, dt * 128:(dt + 1) * 128],
                                    eye_sb)
                nc.vector.tensor_copy(
                    o1T[:, dt * R + m * 128:dt * R + (m + 1) * 128], tps[:])

        # FFN over r-quarters of 256
        for rq in range(4):
            h1s = []
            for ft in range(16):
                p1 = ps.tile([128, 256], F32, tag="mmA", bufs=2)
                for dt in range(4):
                    nc.tensor.matmul(
                        p1[:],
                        w1_all[:, dt * DFF + ft * 128:dt * DFF + (ft + 1) * 128],
                        o1T[:, dt * R + rq * 256:dt * R + (rq + 1) * 256],
                        start=(dt == 0), stop=(dt == 3))
                h1 = wk.tile([128, 256], F32R, tag=f"h1_{ft}", bufs=1)
                nc.scalar.activation(h1[:], p1[:], AF.Relu,
                                     bias=cf[:, O_B1G + ft:O_B1G + ft + 1])
                h1s.append(h1)
            for rm in range(2):
                m = rq * 2 + rm
                p2 = ps.tile([128, D], F32, tag="mmB", bufs=2)
                for ft in range(16):
                    nc.tensor.matmul(
                        p2[:], h1s[ft][:, rm * 128:(rm + 1) * 128],
                        w2_all[:, ft * D:(ft + 1) * D],
                        start=(ft == 0), stop=(ft == 15))
                z2 = wk.tile([128, D], F32, tag="z2")
                nc.vector.tensor_add(z2[:], p2[:], o1_sb[m][:])
                nc.gpsimd.tensor_add(z2[:], z2[:], B2_sb[:])
                yt = wk.tile([128, D], F32, tag="yt")
                layernorm(yt, z2, G2_sb, BE2_sb)
                nc.sync.dma_start(out[m * 128:(m + 1) * 128, :], yt[:])

    nc.compile()
    return nc


def _round_f32r(a):
    b = np.ascontiguousarray(a, dtype=np.float32).view(np.uint32)
    out = (b + 0x7FF + ((b >> 12) & 1)) & np.uint32(0xFFFFF000)
    return out.view(np.float32)


def _pe_table():
    pos = np.arange(S, dtype=np.float32)[:, None]
    div = np.exp(np.arange(0, D, 2, dtype=np.float32) * (-math.log(10000.0) / D))
    ang = pos * div
    pe = np.zeros((S, D), np.float32)
    pe[:, 0::2] = np.sin(ang)
    pe[:, 1::2] = np.cos(ang)
    return pe


def make_in_maps(x, Wq, Wfc, W1, b1, W2, b2, g1, be1, g2, be2):
    f32 = lambda a: np.ascontiguousarray(a, dtype=np.float32)
    xf = f32(x).reshape(S * H * W, D)
    pe = _pe_table()
    M = f32(Wfc).reshape(D, NH, DEP).sum(axis=1).T          # (64, 512)
    Mstk = np.concatenate([M, M], axis=0)                   # (128, 512)

    CF = np.zeros((128, NCF), np.float32)
    CF[:, O_EYE:O_EYE + 128] = np.eye(128, dtype=np.float32)
    CF[:, O_ON1] = 1.0
    CF[:, O_B2:O_B2 + D] = np.tile(f32(b2), (128, 1))
    CF[:, O_G1:O_G1 + D] = np.tile(f32(g1), (128, 1))
    CF[:, O_BE1:O_BE1 + D] = np.tile(f32(be1), (128, 1))
    CF[:, O_G2:O_G2 + D] = np.tile(f32(g2), (128, 1))
    CF[:, O_BE2:O_BE2 + D] = np.tile(f32(be2), (128, 1))
    CF[:, O_B1G:O_B1G + 16] = f32(b1).reshape(16, 128).T

    CR = np.zeros((128, NCR), np.float32)
    CR[:, O_MST:O_MST + D] = _round_f32r(Mstk)
    CR[:, O_ONR] = 1.0
    for jt in range(4):
        for p in range(128):
            CR[2 * jt + p // 64, O_E8 + jt * 128 + p] = 1.0

    shared = dict(
        WqT=_round_f32r(Wq.T), W1T=_round_f32r(W1.T), W2T=_round_f32r(W2.T),
        CF=CF, CR=CR,
    )
    maps = []
    for k in range(NCORES):
        sl = xf[k * R:(k + 1) * R]
        m = dict(shared)
        m["xT"] = _round_f32r(sl.T)
        m["xR"] = np.ascontiguousarray(sl)
        m["peR"] = np.ascontiguousarray(np.repeat(pe[k * 64:(k + 1) * 64], 16, axis=0))
        maps.append(m)
    return maps


def kernel(x, Wq, Wfc, W1, b1, W2, b2, g1, be1, g2, be2, _results_hook=None,
           _trace=False, _tmpdir=None):
    if "nc" not in _cached:
        _cached["nc"] = build_nc()
    nc = _cached["nc"]
    in_maps = make_in_maps(x, Wq, Wfc, W1, b1, W2, b2, g1, be1, g2, be2)
    res = run_bass_kernel_spmd(nc, in_maps, list(range(NCORES)),
                               trace=_trace, tmpdir=_tmpdir)
    if _results_hook is not None:
        _results_hook(res)
    y = np.concatenate([res.results[k]["out"] for k in range(NCORES)], axis=0)
    return y.reshape(S, H, W, D)


# revision 16
# speedup vs baseline: 1.3219x; 1.0621x over previous
"""Trainium2 Bass kernel for nn_EncoderLayer_73315091743398.

The reference module's attention einsums ('hwink,hwijm->hwinm') sum their k/j
indices independently, so the whole attention block collapses to, per
(h,w)-chunk c and head i, over the flat q matrix qf = x@Wq.T + pe viewed as
(8192, 512) in raw (s,h,w) row order:

    u[s]  = sum_d qf[c*512+s, 64i+d]          (segment row sums)
    a     = softmax_s(u)
    v[d]  = sum_s a[s] * qf[c*512+s, 64i+d]
    row   = tile8(v) @ Wfc.T = v @ M,  M[d,:] = sum_b Wfc[:, 64b+d].T

and attn_out viewed (S,H,W,D) has row A[s'] = row_{c=s'//32, i=(s'%32)//4},
independent of (h,w).  Core k owns raw rows [k*1024,(k+1)*1024): these are
exactly attention chunks {2k, 2k+1} AND the residual/FFN rows for
s' in [64k, 64k+64), so the 8 cores run fully independent SPMD programs
(data-parallel over the flat row dimension; no collectives).
"""

import math
import os
import sys
from contextlib import ExitStack

import numpy as np

for _p in ("/opt/trn_rl_repo", "/root/.axon_site/_ro/trn_rl_repo"):
    if os.path.isdir(_p) and _p not in sys.path:
        sys.path.append(_p)

import concourse.bass as bass
import concourse.bacc as bacc
import concourse.mybir as mybir
import concourse.tile as tile
from concourse.bass_utils import run_bass_kernel_spmd

F32 = mybir.dt.float32
F32R = mybir.dt.float32r
AF = mybir.ActivationFunctionType
ALU = mybir.AluOpType
AX = mybir.AxisListType

S, H, W, D = 512, 4, 4, 512
NH, DEP, DFF = 8, 64, 2048
NCORES = 8
R = 1024          # rows per core of the flat (8192, 512) view
EPS = 1e-5

# packed fp32 constant block column offsets
O_EYE, O_ON1, O_B2, O_G1, O_BE1, O_G2, O_BE2, O_B1G = (
    0, 128, 129, 641, 1153, 1665, 2177, 2689)
NCF = 2705
# packed f32r constant block column offsets
O_MST, O_ZER, O_ONR, O_E8 = 0, 512, 528, 529
O_EYR, O_B2R = 1041, 1169
NCR = 1681

_cached = {}


def build_nc():
    """Build the single-core SPMD Bass/Tile program (same program on all 8)."""
    nc = bacc.Bacc("TRN2", debug=False, target_bir_lowering=False)

    xT = nc.dram_tensor("xT", [D, R], F32R, kind="ExternalInput")
    xR = nc.dram_tensor("xR", [R, D], F32, kind="ExternalInput")
    peR = nc.dram_tensor("peR", [R, D], F32, kind="ExternalInput")
    WqT = nc.dram_tensor("WqT", [D, D], F32R, kind="ExternalInput")
    W1T = nc.dram_tensor("W1T", [D, DFF], F32R, kind="ExternalInput")
    W2T = nc.dram_tensor("W2T", [DFF, D], F32R, kind="ExternalInput")
    CF = nc.dram_tensor("CF", [128, NCF], F32, kind="ExternalInput")
    CR = nc.dram_tensor("CR", [128, NCR], F32R, kind="ExternalInput")
    out = nc.dram_tensor("out", [R, D], F32, kind="ExternalOutput")

    with ExitStack() as ctx:
        tc = ctx.enter_context(tile.TileContext(nc))
        cst = ctx.enter_context(tc.tile_pool(name="cst", bufs=1))
        xp = ctx.enter_context(tc.tile_pool(name="xp", bufs=1))
        qp = ctx.enter_context(tc.tile_pool(name="qp", bufs=1))
        wk = ctx.enter_context(tc.tile_pool(name="wk", bufs=2))
        ps = ctx.enter_context(tc.tile_pool(name="ps", bufs=1, space="PSUM"))

        # ---- loads, cheapest-needed-first so PE can start early ----
        xT_all = xp.tile([128, 4 * R], F32R, tag="dT", name="xT_all")
        wq_all = cst.tile([128, 4 * D], F32R, tag="wq", name="wq_all")
        for dt in range(4):
            nc.sync.dma_start(xT_all[:, dt * R:(dt + 1) * R],
                              xT[dt * 128:(dt + 1) * 128, :])
            nc.sync.dma_start(wq_all[:, dt * D:(dt + 1) * D],
                              WqT[dt * 128:(dt + 1) * 128, :])
        pe_all = xp.tile([128, 8 * D], F32, tag="peA", name="pe_all")
        nc.sync.dma_start(pe_all[:].rearrange("p (m d) -> p m d", m=8),
                          peR.rearrange("(m p) d -> p m d", p=128))
        cf = cst.tile([128, NCF], F32, tag="cf", name="cf")
        nc.sync.dma_start(cf[:], CF[:])
        cr = cst.tile([128, NCR], F32R, tag="cr", name="cr")
        nc.sync.dma_start(cr[:], CR[:])
        w1_all = cst.tile([128, 4 * DFF], F32R, tag="w1", name="w1_all")
        nc.sync.dma_start(w1_all[:].rearrange("p (t j) -> p t j", t=4),
                          W1T.rearrange("(t p) j -> p t j", p=128))
        w2_all = cst.tile([128, 16 * D], F32R, tag="w2", name="w2_all")
        nc.sync.dma_start(w2_all[:].rearrange("p (t j) -> p t j", t=16),
                          W2T.rearrange("(t p) j -> p t j", p=128))

        eye_sb = cf[:, O_EYE:O_EYE + 128]
        on1r = cr[:, O_ONR:O_ONR + 1]
        Mst_sb = cr[:, O_MST:O_MST + D]
        zer8 = cr[:, O_ZER:O_ZER + 8]
        B2_sb = cf[:, O_B2:O_B2 + D]
        G1_sb = cf[:, O_G1:O_G1 + D]
        BE1_sb = cf[:, O_BE1:O_BE1 + D]
        G2_sb = cf[:, O_G2:O_G2 + D]
        BE2_sb = cf[:, O_BE2:O_BE2 + D]
        epsT = cst.tile([128, 1], F32, tag="eps", name="epsT")
        nc.vector.memset(epsT[:], EPS)

        q_sb = [qp.tile([128, D], F32, tag=f"q{m}", name=f"qsb{m}") for m in range(8)]
        o1_sb = [qp.tile([128, D], F32, tag=f"o1{m}", name=f"o1sb{m}") for m in range(8)]
        uT = [qp.tile([8, D], F32, tag=f"uT{c}", name=f"uTsb{c}") for c in range(2)]

        def layernorm(dst, zin, g_t, be_t):
            """dst = LN(zin) * g + be for a 128-row tile (zin SBUF f32)."""
            ssum = wk.tile([128, 1], F32, tag="ls")
            nc.vector.tensor_reduce(ssum[:], zin[:], axis=AX.X, op=ALU.add)
            mu = wk.tile([128, 1], F32, tag="lm")
            nc.vector.tensor_scalar_mul(mu[:], ssum[:], 1.0 / D)
            sqd = wk.tile([128, D], F32, tag="lq", bufs=1)
            ssq = wk.tile([128, 1], F32, tag="lsq")
            nc.scalar.activation(sqd[:], zin[:], AF.Square, accum_out=ssq[:])
            msq = wk.tile([128, 1], F32, tag="lms")
            nc.vector.tensor_scalar_mul(msq[:], ssq[:], 1.0 / D)
            mu2 = wk.tile([128, 1], F32, tag="lm2")
            nc.vector.tensor_mul(mu2[:], mu[:], mu[:])
            var = wk.tile([128, 1], F32, tag="lv")
            nc.vector.tensor_sub(var[:], msq[:], mu2[:])
            sd = wk.tile([128, 1], F32, tag="lsd")
            nc.scalar.activation(sd[:], var[:], AF.Sqrt, bias=epsT[:, :])
            rsd = wk.tile([128, 1], F32, tag="lr")
            nc.vector.reciprocal(rsd[:], sd[:])
            nrm = wk.tile([128, D], F32, tag="ln", bufs=1)
            nc.vector.tensor_scalar(nrm[:], zin[:], mu[:], rsd[:],
                                    op0=ALU.subtract, op1=ALU.mult)
            nc.vector.tensor_mul(nrm[:], nrm[:], g_t[:])
            nc.vector.tensor_add(dst[:], nrm[:], be_t[:])

        def q_stage(m):
            qps = ps.tile([128, D], F32, tag="mmA", bufs=2)
            for dt in range(4):
                nc.tensor.matmul(
                    qps[:], xT_all[:, dt * R + m * 128:dt * R + (m + 1) * 128],
                    wq_all[:, dt * D:(dt + 1) * D],
                    start=(dt == 0), stop=(dt == 3))
            nc.vector.tensor_add(q_sb[m][:], qps[:],
                                 pe_all[:, m * D:(m + 1) * D])
            useg = wk.tile([128, 8], F32, tag="useg")
            nc.vector.tensor_reduce(
                useg[:], q_sb[m][:].rearrange("p (h d) -> p h d", h=8),
                axis=AX.X, op=ALU.add)
            utp = ps.tile([8, 128], F32, tag="tp", bufs=2)
            nc.tensor.transpose(utp[:], useg[:], eye_sb)
            c, st = divmod(m, 4)
            nc.vector.tensor_copy(uT[c][:, st * 128:(st + 1) * 128], utp[:])

        def attn_stage(c):
            # softmax over s for the 8 heads of chunk c
            mx = wk.tile([8, 1], F32, tag="mx")
            nc.vector.tensor_reduce(mx[:], uT[c][:], axis=AX.X, op=ALU.max)
            nmx = wk.tile([8, 1], F32, tag="nmx")
            nc.vector.tensor_scalar_mul(nmx[:], mx[:], -1.0)
            ex = wk.tile([8, D], F32, tag="ex", bufs=1)
            ssum = wk.tile([8, 1], F32, tag="esum")
            nc.scalar.activation(ex[:], uT[c][:], AF.Exp, bias=nmx[:, :],
                                 accum_out=ssum[:])
            rcp = wk.tile([8, 1], F32, tag="ercp")
            nc.vector.reciprocal(rcp[:], ssum[:])
            a_t = ex
            nc.vector.tensor_scalar_mul(a_t[:], ex[:], rcp[:])

            # prod[st] = q tile * broadcast(aT); vcat = column sums via ones-mm
            vca = ps.tile([1, D], F32, tag="vc", bufs=2)
            for st in range(4):
                atp = ps.tile([128, 8], F32, tag="tp", bufs=2)
                nc.tensor.transpose(atp[:], a_t[:, st * 128:(st + 1) * 128],
                                    eye_sb[:8, :8])
                aTs = wk.tile([128, 8], F32, tag=f"aT{st}", bufs=1)
                nc.vector.tensor_copy(aTs[:], atp[:])
                prod = wk.tile([128, D], F32R, tag=f"prod{st}", bufs=1)
                nc.vector.tensor_tensor(
                    prod[:].rearrange("p (h d) -> p h d", h=8),
                    q_sb[c * 4 + st][:].rearrange("p (h d) -> p h d", h=8),
                    aTs[:].unsqueeze(-1).broadcast_to([128, 8, 64]),
                    op=ALU.mult)
                nc.tensor.matmul(vca[:], on1r, prod[:],
                                 start=(st == 0), stop=(st == 3))
            vcs = wk.tile([1, D], F32, tag="vcs", bufs=1)
            nc.vector.tensor_copy(vcs[:], vca[:])

            # Vm[p, 2t+e] = vcat[128t+p] * (p//64 == e);  La = Vm.T @ Mstk
            vm = wk.tile([128, 8], F32R, tag="vm")
            nc.vector.tensor_copy(vm[:], zer8)
            for tt in range(4):
                vtp = ps.tile([128, 1], F32, tag="tp", bufs=2)
                nc.tensor.transpose(vtp[:], vcs[:, tt * 128:(tt + 1) * 128],
                                    eye_sb[:1, :1])
                nc.vector.tensor_copy(vm[0:64, 2 * tt:2 * tt + 1], vtp[0:64, :])
                nc.vector.tensor_copy(vm[64:128, 2 * tt + 1:2 * tt + 2],
                                      vtp[64:128, :])
            lap = ps.tile([8, D], F32, tag="vc", bufs=2)
            nc.tensor.matmul(lap[:], vm[:], Mst_sb, start=True, stop=True)
            las = wk.tile([8, D], F32R, tag="las")
            nc.vector.tensor_copy(las[:], lap[:])

            for jt in range(4):
                m = c * 4 + jt
                bcp = ps.tile([128, D], F32, tag="mmB", bufs=2)
                nc.tensor.matmul(bcp[:],
                                 cr[0:8, O_E8 + jt * 128:O_E8 + (jt + 1) * 128],
                                 las[:], start=True, stop=True)
                xrt = wk.tile([128, D], F32, tag="xr")
                nc.gpsimd.dma_start(xrt[:], xR[m * 128:(m + 1) * 128, :])
                z1 = wk.tile([128, D], F32, tag="z1", bufs=1)
                nc.vector.tensor_add(z1[:], bcp[:], xrt[:])
                layernorm(o1_sb[m], z1, G1_sb, BE1_sb)

        o1T = xp.tile([128, 4 * R], F32R, tag="dT", name="o1T_all")

        def trans_stage(m):
            for dt in range(4):
                tps = ps.tile([128, 128], F32, tag="tp", bufs=2)
                nc.tensor.transpose(tps[:], o1_sb[m][:, dt * 128:(dt + 1) * 128],
                                    eye_sb)
                nc.vector.tensor_copy(
                    o1T[:, dt * R + m * 128:dt * R + (m + 1) * 128], tps[:])

        def ffn_quarter(rq):
            h1s = []
            for ft in range(16):
                p1 = ps.tile([128, 256], F32, tag="mmA", bufs=2)
                for dt in range(4):
                    nc.tensor.matmul(
                        p1[:],
                        w1_all[:, dt * DFF + ft * 128:dt * DFF + (ft + 1) * 128],
                        o1T[:, dt * R + rq * 256:dt * R + (rq + 1) * 256],
                        start=(dt == 0), stop=(dt == 3))
                h1 = wk.tile([128, 256], F32R, tag=f"h1_{ft}", bufs=1)
                nc.scalar.activation(h1[:], p1[:], AF.Relu,
                                     bias=cf[:, O_B1G + ft:O_B1G + ft + 1])
                h1s.append(h1)
            for rm in range(2):
                m = rq * 2 + rm
                p2 = ps.tile([128, D], F32, tag="mmB", bufs=2)
                for ft in range(16):
                    nc.tensor.matmul(
                        p2[:], h1s[ft][:, rm * 128:(rm + 1) * 128],
                        w2_all[:, ft * D:(ft + 1) * D],
                        start=(ft == 0), stop=False)
                nc.tensor.matmul(p2[:], cr[:, O_EYR:O_EYR + 128],
                                 cr[:, O_B2R:O_B2R + D],
                                 start=False, stop=True)
                z2 = wk.tile([128, D], F32, tag="z2", bufs=1)
                nc.vector.tensor_add(z2[:], p2[:], o1_sb[m][:])
                yt = wk.tile([128, D], F32, tag="yt")
                layernorm(yt, z2, G2_sb, BE2_sb)
                nc.sync.dma_start(out[m * 128:(m + 1) * 128, :], yt[:])

        for m in range(8):
            q_stage(m)
        attn_stage(0)
        for m in range(4):
            trans_stage(m)
        attn_stage(1)
        ffn_quarter(0)
        ffn_quarter(1)
        for m in range(4, 8):
            trans_stage(m)
        ffn_quarter(2)
        ffn_quarter(3)

    nc.compile()
    return nc


def _round_f32r(a):
    b = np.ascontiguousarray(a, dtype=np.float32).view(np.uint32)
    out = (b + 0x7FF + ((b >> 12) & 1)) & np.uint32(0xFFFFF000)
    return out.view(np.float32)


def _pe_table():
    pos = np.arange(S, dtype=np.float32)[:, None]
    div = np.exp(np.arange(0, D, 2, dtype=np.float32) * (-math.log(10000.0) / D))
    ang = pos * div
    pe = np.zeros((S, D), np.float32)
    pe[:, 0::2] = np.sin(ang)
    pe[:, 1::2] = np.cos(ang)
    return pe


def make_in_maps(x, Wq, Wfc, W1, b1, W2, b2, g1, be1, g2, be2):
    f32 = lambda a: np.ascontiguousarray(a, dtype=np.float32)
    xf = f32(x).reshape(S * H * W, D)
    pe = _pe_table()
    M = f32(Wfc).reshape(D, NH, DEP).sum(axis=1).T          # (64, 512)
    Mstk = np.concatenate([M, M], axis=0)                   # (128, 512)

    CF = np.zeros((128, NCF), np.float32)
    CF[:, O_EYE:O_EYE + 128] = np.eye(128, dtype=np.float32)
    CF[:, O_ON1] = 1.0
    CF[:, O_B2:O_B2 + D] = np.tile(f32(b2), (128, 1))
    CF[:, O_G1:O_G1 + D] = np.tile(f32(g1), (128, 1))
    CF[:, O_BE1:O_BE1 + D] = np.tile(f32(be1), (128, 1))
    CF[:, O_G2:O_G2 + D] = np.tile(f32(g2), (128, 1))
    CF[:, O_BE2:O_BE2 + D] = np.tile(f32(be2), (128, 1))
    CF[:, O_B1G:O_B1G + 16] = f32(b1).reshape(16, 128).T

    CR = np.zeros((128, NCR), np.float32)
    CR[:, O_MST:O_MST + D] = _round_f32r(Mstk)
    CR[:, O_ONR] = 1.0
    CR[:, O_EYR:O_EYR + 128] = np.eye(128, dtype=np.float32)
    CR[:, O_B2R:O_B2R + D] = _round_f32r(np.tile(f32(b2), (128, 1)))
    for jt in range(4):
        for p in range(128):
            CR[2 * jt + p // 64, O_E8 + jt * 128 + p] = 1.0

    shared = dict(
        WqT=_round_f32r(Wq.T), W1T=_round_f32r(W1.T), W2T=_round_f32r(W2.T),
        CF=CF, CR=CR,
    )
    maps = []
    for k in range(NCORES):
        sl = xf[k * R:(k + 1) * R]
        m = dict(shared)
        m["xT"] = _round_f32r(sl.T)
        m["xR"] = np.ascontiguousarray(sl)
        m["peR"] = np.ascontiguousarray(np.repeat(pe[k * 64:(k + 1) * 64], 16, axis=0))
        maps.append(m)
    return maps


def kernel(x, Wq, Wfc, W1, b1, W2, b2, g1, be1, g2, be2, _results_hook=None,
           _trace=False, _tmpdir=None):
    if "nc" not in _cached:
        _cached["nc"] = build_nc()
    nc = _cached["nc"]
    in_maps = make_in_maps(x, Wq, Wfc, W1, b1, W2, b2, g1, be1, g2, be2)
    res = run_bass_kernel_spmd(nc, in_maps, list(range(NCORES)),
                               trace=_trace, tmpdir=_tmpdir)
    if _results_hook is not None:
        _results_hook(res)
    y = np.concatenate([res.results[k]["out"] for k in range(NCORES)], axis=0)
    return y.reshape(S, H, W, D)


# revision 19
# speedup vs baseline: 1.3797x; 1.0437x over previous
"""Trainium2 Bass kernel for nn_EncoderLayer_73315091743398.

The reference module's attention einsums ('hwink,hwijm->hwinm') sum their k/j
indices independently, so the whole attention block collapses to, per
(h,w)-chunk c and head i, over the flat q matrix qf = x@Wq.T + pe viewed as
(8192, 512) in raw (s,h,w) row order:

    u[s]  = sum_d qf[c*512+s, 64i+d]          (segment row sums)
    a     = softmax_s(u)
    v[d]  = sum_s a[s] * qf[c*512+s, 64i+d]
    row   = tile8(v) @ Wfc.T = v @ M,  M[d,:] = sum_b Wfc[:, 64b+d].T

and attn_out viewed (S,H,W,D) has row A[s'] = row_{c=s'//32, i=(s'%32)//4},
independent of (h,w).  Core k owns raw rows [k*1024,(k+1)*1024): these are
exactly attention chunks {2k, 2k+1} AND the residual/FFN rows for
s' in [64k, 64k+64), so the 8 cores run fully independent SPMD programs
(data-parallel over the flat row dimension; no collectives).
"""

import math
import os
import sys
from contextlib import ExitStack

import numpy as np

for _p in ("/opt/trn_rl_repo", "/root/.axon_site/_ro/trn_rl_repo"):
    if os.path.isdir(_p) and _p not in sys.path:
        sys.path.append(_p)

import concourse.bass as bass
import concourse.bacc as bacc
import concourse.mybir as mybir
import concourse.tile as tile
from concourse.bass_utils import run_bass_kernel_spmd

F32 = mybir.dt.float32
F32R = mybir.dt.float32r
AF = mybir.ActivationFunctionType
ALU = mybir.AluOpType
AX = mybir.AxisListType

S, H, W, D = 512, 4, 4, 512
NH, DEP, DFF = 8, 64, 2048
NCORES = 8
R = 1024          # rows per core of the flat (8192, 512) view
EPS = 1e-5

# packed fp32 constant block column offsets
O_EYE, O_ON1, O_B2, O_G1, O_BE1, O_G2, O_BE2, O_B1G = (
    0, 128, 129, 641, 1153, 1665, 2177, 2689)
O_G1C, O_BE1C = 2705, 2709
NCF = 2713
# packed f32r constant block column offsets
O_MST, O_ZER, O_ONR, O_E8 = 0, 512, 528, 529
O_EYR, O_B2R = 1041, 1169
NCR = 1681

_cached = {}


def build_nc():
    """Build the single-core SPMD Bass/Tile program (same program on all 8)."""
    nc = bacc.Bacc("TRN2", debug=False, target_bir_lowering=False)

    xT = nc.dram_tensor("xT", [D, R], F32R, kind="ExternalInput")
    xR = nc.dram_tensor("xR", [R, D], F32, kind="ExternalInput")
    peR = nc.dram_tensor("peR", [R, D], F32, kind="ExternalInput")
    WqT = nc.dram_tensor("WqT", [D, D], F32R, kind="ExternalInput")
    W1T = nc.dram_tensor("W1T", [D, DFF], F32R, kind="ExternalInput")
    W2T = nc.dram_tensor("W2T", [DFF, D], F32R, kind="ExternalInput")
    CF = nc.dram_tensor("CF", [128, NCF], F32, kind="ExternalInput")
    CR = nc.dram_tensor("CR", [128, NCR], F32R, kind="ExternalInput")
    out = nc.dram_tensor("out", [R, D], F32, kind="ExternalOutput")

    with ExitStack() as ctx:
        tc = ctx.enter_context(tile.TileContext(nc))
        cst = ctx.enter_context(tc.tile_pool(name="cst", bufs=1))
        xp = ctx.enter_context(tc.tile_pool(name="xp", bufs=1))
        qp = ctx.enter_context(tc.tile_pool(name="qp", bufs=1))
        wk = ctx.enter_context(tc.tile_pool(name="wk", bufs=2))
        ps = ctx.enter_context(tc.tile_pool(name="ps", bufs=1, space="PSUM"))

        # ---- loads, cheapest-needed-first so PE can start early ----
        xT_all = xp.tile([128, 4 * R], F32R, tag="dT", name="xT_all")
        wq_all = cst.tile([128, 4 * D], F32R, tag="wq", name="wq_all")
        for dt in range(4):
            nc.sync.dma_start(wq_all[:, dt * D:(dt + 1) * D],
                              WqT[dt * 128:(dt + 1) * 128, :])
        xTv = xT.rearrange("(t p) r -> p t r", p=128)
        xTd = xT_all[:].rearrange("p (t r) -> p t r", t=4)
        for m in range(8):
            nc.sync.dma_start(xTd[:, :, m * 128:(m + 1) * 128],
                              xTv[:, :, m * 128:(m + 1) * 128])
        pe_all = xp.tile([128, 8 * D], F32, tag="peA", name="pe_all")
        nc.sync.dma_start(pe_all[:].rearrange("p (m d) -> p m d", m=8),
                          peR.rearrange("(m p) d -> p m d", p=128))
        cf = cst.tile([128, NCF], F32, tag="cf", name="cf")
        nc.sync.dma_start(cf[:], CF[:])
        cr = cst.tile([128, NCR], F32R, tag="cr", name="cr")
        nc.sync.dma_start(cr[:], CR[:])
        w1_all = cst.tile([128, 4 * DFF], F32R, tag="w1", name="w1_all")
        nc.sync.dma_start(w1_all[:].rearrange("p (t j) -> p t j", t=4),
                          W1T.rearrange("(t p) j -> p t j", p=128))
        w2_all = cst.tile([128, 16 * D], F32R, tag="w2", name="w2_all")
        nc.sync.dma_start(w2_all[:].rearrange("p (t j) -> p t j", t=16),
                          W2T.rearrange("(t p) j -> p t j", p=128))

        eye_sb = cf[:, O_EYE:O_EYE + 128]
        on1r = cr[:, O_ONR:O_ONR + 1]
        Mst_sb = cr[:, O_MST:O_MST + D]
        zer8 = cr[:, O_ZER:O_ZER + 8]
        B2_sb = cf[:, O_B2:O_B2 + D]
        G1_sb = cf[:, O_G1:O_G1 + D]
        BE1_sb = cf[:, O_BE1:O_BE1 + D]
        G2_sb = cf[:, O_G2:O_G2 + D]
        BE2_sb = cf[:, O_BE2:O_BE2 + D]
        epsT = cst.tile([128, 1], F32, tag="eps", name="epsT")
        nc.vector.memset(epsT[:], EPS)

        q_sb = [qp.tile([128, D], F32, tag=f"q{m}", name=f"qsb{m}") for m in range(8)]
        o1_sb = [qp.tile([128, D], F32, tag=f"o1{m}", name=f"o1sb{m}") for m in range(8)]
        uT = [qp.tile([8, D], F32, tag=f"uT{c}", name=f"uTsb{c}") for c in range(2)]

        def layernorm(dst, zin, g_t, be_t):
            """dst = LN(zin) * g + be for a 128-row tile (zin SBUF f32)."""
            ssum = wk.tile([128, 1], F32, tag="ls")
            nc.vector.tensor_reduce(ssum[:], zin[:], axis=AX.X, op=ALU.add)
            mu = wk.tile([128, 1], F32, tag="lm")
            nc.vector.tensor_scalar_mul(mu[:], ssum[:], 1.0 / D)
            sqd = wk.tile([128, D], F32, tag="lq", bufs=1)
            ssq = wk.tile([128, 1], F32, tag="lsq")
            nc.scalar.activation(sqd[:], zin[:], AF.Square, accum_out=ssq[:])
            msq = wk.tile([128, 1], F32, tag="lms")
            nc.vector.tensor_scalar_mul(msq[:], ssq[:], 1.0 / D)
            mu2 = wk.tile([128, 1], F32, tag="lm2")
            nc.vector.tensor_mul(mu2[:], mu[:], mu[:])
            var = wk.tile([128, 1], F32, tag="lv")
            nc.vector.tensor_sub(var[:], msq[:], mu2[:])
            sd = wk.tile([128, 1], F32, tag="lsd")
            nc.scalar.activation(sd[:], var[:], AF.Sqrt, bias=epsT[:, :])
            rsd = wk.tile([128, 1], F32, tag="lr")
            nc.vector.reciprocal(rsd[:], sd[:])
            nrm = wk.tile([128, D], F32, tag="ln", bufs=1)
            nc.vector.tensor_scalar(nrm[:], zin[:], mu[:], rsd[:],
                                    op0=ALU.subtract, op1=ALU.mult)
            if be_t is None:
                nc.vector.tensor_mul(dst[:], nrm[:], g_t[:])
            else:
                nc.vector.tensor_mul(nrm[:], nrm[:], g_t[:])
                nc.vector.tensor_add(dst[:], nrm[:], be_t[:])

        def q_stage(m):
            qps = ps.tile([128, D], F32, tag="mmA", bufs=2)
            for dt in range(4):
                nc.tensor.matmul(
                    qps[:], xT_all[:, dt * R + m * 128:dt * R + (m + 1) * 128],
                    wq_all[:, dt * D:(dt + 1) * D],
                    start=(dt == 0), stop=(dt == 3))
            nc.vector.tensor_add(q_sb[m][:], qps[:],
                                 pe_all[:, m * D:(m + 1) * D])
            useg = wk.tile([128, 8], F32, tag="useg")
            nc.vector.tensor_reduce(
                useg[:], q_sb[m][:].rearrange("p (h d) -> p h d", h=8),
                axis=AX.X, op=ALU.add)
            utp = ps.tile([8, 128], F32, tag="tp", bufs=2)
            nc.tensor.transpose(utp[:], useg[:], eye_sb)
            c, st = divmod(m, 4)
            nc.vector.tensor_copy(uT[c][:, st * 128:(st + 1) * 128], utp[:])

        def attn_softmax(c):
            mx = wk.tile([8, 1], F32, tag="mx")
            nc.vector.tensor_reduce(mx[:], uT[c][:], axis=AX.X, op=ALU.max)
            nmx = wk.tile([8, 1], F32, tag="nmx")
            nc.vector.tensor_scalar_mul(nmx[:], mx[:], -1.0)
            ex = wk.tile([8, D], F32, tag=f"ex{c}", bufs=1)
            ssum = wk.tile([8, 1], F32, tag="esum")
            nc.scalar.activation(ex[:], uT[c][:], AF.Exp, bias=nmx[:, :],
                                 accum_out=ssum[:])
            rcp = wk.tile([8, 1], F32, tag="ercp")
            nc.vector.reciprocal(rcp[:], ssum[:])
            nc.vector.tensor_scalar_mul(ex[:], ex[:], rcp[:])
            return ex

        def attn_prods(c, a_t):
            vca = ps.tile([1, D], F32, tag="vc", bufs=2)
            for st in range(4):
                atp = ps.tile([128, 8], F32, tag="tp", bufs=2)
                nc.tensor.transpose(atp[:], a_t[:, st * 128:(st + 1) * 128],
                                    eye_sb[:8, :8])
                aTs = wk.tile([128, 8], F32, tag=f"aT{st}", bufs=1)
                nc.vector.tensor_copy(aTs[:], atp[:])
                prod = wk.tile([128, D], F32R, tag=f"prod{st}", bufs=1)
                nc.vector.tensor_tensor(
                    prod[:].rearrange("p (h d) -> p h d", h=8),
                    q_sb[c * 4 + st][:].rearrange("p (h d) -> p h d", h=8),
                    aTs[:].unsqueeze(-1).broadcast_to([128, 8, 64]),
                    op=ALU.mult)
                nc.tensor.matmul(vca[:], on1r, prod[:],
                                 start=(st == 0), stop=(st == 3))
            return vca

        def attn_la(c, vca):
            vcs = wk.tile([1, D], F32, tag="vcs", bufs=1)
            nc.vector.tensor_copy(vcs[:], vca[:])
            vm = wk.tile([128, 8], F32R, tag="vm")
            nc.vector.tensor_copy(vm[:], zer8)
            for tt in range(4):
                vtp = ps.tile([128, 1], F32, tag="tp", bufs=2)
                nc.tensor.transpose(vtp[:], vcs[:, tt * 128:(tt + 1) * 128],
                                    eye_sb[:1, :1])
                nc.vector.tensor_copy(vm[0:64, 2 * tt:2 * tt + 1], vtp[0:64, :])
                nc.vector.tensor_copy(vm[64:128, 2 * tt + 1:2 * tt + 2],
                                      vtp[64:128, :])
            lap = ps.tile([8, D], F32, tag="vc", bufs=2)
            nc.tensor.matmul(lap[:], vm[:], Mst_sb, start=True, stop=True)
            las = wk.tile([8, D], F32R, tag=f"las{c}", bufs=1)
            nc.vector.tensor_copy(las[:], lap[:])
            return las

        def attn_resid(c, las, jt):
            m = c * 4 + jt
            bcp = ps.tile([128, D], F32, tag="mmB", bufs=2)
            nc.tensor.matmul(bcp[:],
                             cr[0:8, O_E8 + jt * 128:O_E8 + (jt + 1) * 128],
                             las[:], start=True, stop=True)
            xrt = wk.tile([128, D], F32, tag="xr")
            nc.gpsimd.dma_start(xrt[:], xR[m * 128:(m + 1) * 128, :])
            z1 = wk.tile([128, D], F32, tag="z1", bufs=1)
            nc.vector.tensor_add(z1[:], bcp[:], xrt[:])
            layernorm(o1_sb[m], z1, G1_sb, None)

        o1T = xp.tile([128, 4 * R], F32R, tag="dT", name="o1T_all")

        def trans_stage(m):
            for dt in range(4):
                tps = ps.tile([128, 128], F32, tag="tp", bufs=2)
                nc.tensor.transpose(tps[:], o1_sb[m][:, dt * 128:(dt + 1) * 128],
                                    eye_sb)
                nc.vector.tensor_scalar(
                    o1T[:, dt * R + m * 128:dt * R + (m + 1) * 128], tps[:],
                    cf[:, O_BE1C + dt:O_BE1C + dt + 1], None, op0=ALU.add)

        h1store = {}

        def ffn_h1(rq):
            h1s = []
            for ft in range(16):
                p1 = ps.tile([128, 256], F32, tag="mmA", bufs=2)
                for dt in range(4):
                    nc.tensor.matmul(
                        p1[:],
                        w1_all[:, dt * DFF + ft * 128:dt * DFF + (ft + 1) * 128],
                        o1T[:, dt * R + rq * 256:dt * R + (rq + 1) * 256],
                        start=(dt == 0), stop=(dt == 3))
                h1 = wk.tile([128, 256], F32R, tag=f"h1_{ft}", bufs=1)
                nc.scalar.activation(h1[:], p1[:], AF.Relu,
                                     bias=cf[:, O_B1G + ft:O_B1G + ft + 1])
                h1s.append(h1)
            h1store[rq] = h1s

        def ffn_rm(rq):
            h1s = h1store[rq]
            for rm in range(2):
                m = rq * 2 + rm
                p2 = ps.tile([128, D], F32, tag="mmB", bufs=2)
                for ft in range(16):
                    nc.tensor.matmul(
                        p2[:], h1s[ft][:, rm * 128:(rm + 1) * 128],
                        w2_all[:, ft * D:(ft + 1) * D],
                        start=(ft == 0), stop=False)
                nc.tensor.matmul(p2[:], cr[:, O_EYR:O_EYR + 128],
                                 cr[:, O_B2R:O_B2R + D],
                                 start=False, stop=True)
                z2 = wk.tile([128, D], F32, tag="z2", bufs=1)
                nc.vector.tensor_add(z2[:], p2[:], o1_sb[m][:])
                yt = wk.tile([128, D], F32, tag="yt", bufs=1)
                layernorm(yt, z2, G2_sb, BE2_sb)
                nc.sync.dma_start(out[m * 128:(m + 1) * 128, :], yt[:])

        for m in range(4):
            q_stage(m)
        a0 = attn_softmax(0)
        q_stage(4)
        vca0 = attn_prods(0, a0)
        q_stage(5)
        q_stage(6)
        las0 = attn_la(0, vca0)
        q_stage(7)
        for jt in range(4):
            attn_resid(0, las0, jt)
        a1 = attn_softmax(1)
        for m in range(4):
            trans_stage(m)
        vca1 = attn_prods(1, a1)
        las1 = attn_la(1, vca1)
        ffn_h1(0)
        attn_resid(1, las1, 0)
        attn_resid(1, las1, 1)
        ffn_rm(0)
        attn_resid(1, las1, 2)
        attn_resid(1, las1, 3)
        ffn_h1(1)
        ffn_rm(1)
        trans_stage(4)
        trans_stage(5)
        ffn_h1(2)
        trans_stage(6)
        trans_stage(7)
        ffn_rm(2)
        ffn_h1(3)
        ffn_rm(3)

    nc.compile()
    return nc


def _round_f32r(a):
    b = np.ascontiguousarray(a, dtype=np.float32).view(np.uint32)
    out = (b + 0x7FF + ((b >> 12) & 1)) & np.uint32(0xFFFFF000)
    return out.view(np.float32)


def _pe_table():
    pos = np.arange(S, dtype=np.float32)[:, None]
    div = np.exp(np.arange(0, D, 2, dtype=np.float32) * (-math.log(10000.0) / D))
    ang = pos * div
    pe = np.zeros((S, D), np.float32)
    pe[:, 0::2] = np.sin(ang)
    pe[:, 1::2] = np.cos(ang)
    return pe


def make_in_maps(x, Wq, Wfc, W1, b1, W2, b2, g1, be1, g2, be2):
    f32 = lambda a: np.ascontiguousarray(a, dtype=np.float32)
    xf = f32(x).reshape(S * H * W, D)
    pe = _pe_table()
    M = f32(Wfc).reshape(D, NH, DEP).sum(axis=1).T          # (64, 512)
    Mstk = np.concatenate([M, M], axis=0)                   # (128, 512)

    CF = np.zeros((128, NCF), np.float32)
    CF[:, O_EYE:O_EYE + 128] = np.eye(128, dtype=np.float32)
    CF[:, O_ON1] = 1.0
    CF[:, O_B2:O_B2 + D] = np.tile(f32(b2), (128, 1))
    CF[:, O_G1:O_G1 + D] = np.tile(f32(g1), (128, 1))
    CF[:, O_BE1:O_BE1 + D] = np.tile(f32(be1), (128, 1))
    CF[:, O_G2:O_G2 + D] = np.tile(f32(g2), (128, 1))
    CF[:, O_BE2:O_BE2 + D] = np.tile(f32(be2), (128, 1))
    CF[:, O_B1G:O_B1G + 16] = f32(b1).reshape(16, 128).T
    CF[:, O_G1C:O_G1C + 4] = f32(g1).reshape(4, 128).T
    CF[:, O_BE1C:O_BE1C + 4] = f32(be1).reshape(4, 128).T

    CR = np.zeros((128, NCR), np.float32)
    CR[:, O_MST:O_MST + D] = _round_f32r(Mstk)
    CR[:, O_ONR] = 1.0
    CR[:, O_EYR:O_EYR + 128] = np.eye(128, dtype=np.float32)
    CR[:, O_B2R:O_B2R + D] = _round_f32r(np.tile(f32(b2) + f32(be1), (128, 1)))
    for jt in range(4):
        for p in range(128):
            CR[2 * jt + p // 64, O_E8 + jt * 128 + p] = 1.0

    shared = dict(
        WqT=_round_f32r(Wq.T), W1T=_round_f32r(W1.T), W2T=_round_f32r(W2.T),
        CF=CF, CR=CR,
    )
    maps = []
    for k in range(NCORES):
        sl = xf[k * R:(k + 1) * R]
        m = dict(shared)
        m["xT"] = _round_f32r(sl.T)
        m["xR"] = np.ascontiguousarray(sl)
        m["peR"] = np.ascontiguousarray(np.repeat(pe[k * 64:(k + 1) * 64], 16, axis=0))
        maps.append(m)
    return maps


def kernel(x, Wq, Wfc, W1, b1, W2, b2, g1, be1, g2, be2, _results_hook=None,
           _trace=False, _tmpdir=None):
    if "nc" not in _cached:
        _cached["nc"] = build_nc()
    nc = _cached["nc"]
    in_maps = make_in_maps(x, Wq, Wfc, W1, b1, W2, b2, g1, be1, g2, be2)
    res = run_bass_kernel_spmd(nc, in_maps, list(range(NCORES)),
                               trace=_trace, tmpdir=_tmpdir)
    if _results_hook is not None:
        _results_hook(res)
    y = np.concatenate([res.results[k]["out"] for k in range(NCORES)], axis=0)
    return y.reshape(S, H, W, D)
